# revision 1
# baseline (speedup 1.0000x reference)
"""Trainium2 Bass kernel for CrossGraphAttention (ragged per-graph MHA + linear).

Strategy: data-parallel over graphs (2 graphs per core x 8 cores). All graphs
padded to a common n_pad (multiple of 128). Per core the device program:
  1. QKV projection: all qk^T row-tiles + V natural per q-block (pass 1),
     then pure attention + output projection per q-block (pass 2).
  2. Scores computed TRANSPOSED (S^T[k, q]) per head-pair so the softmax
     denominator is a ones-vector matmul (partition reductions are
     PE-friendly); exp fused with the PSUM->SBUF eviction on the scalar
     engine ([128, 2*w] per instruction), with key-padding masking via a
     per-partition bias of -1e30 (exp -> 0). Head-pair score tiles are
     double-buffered in PSUM so PE and ACT pipeline across k-tiles. The
     query iteration covers only ceil(max_graph/64)*64 columns, not the
     128-multiple k-layout stride.
  3. ctx^T accumulated over k-tiles in PSUM (2 heads packed per bank via
     column tiling, fp16 operands; the exp bias also folds in a fixed -8
     offset so probabilities stay in fp16 range — it cancels in softmax);
     normalization by 1/denom applied via a rank-1 broadcast matmul +
     vector multiply.
  4. Fused output projection y = ctx @ (lin_w @ out_proj_w)^T.
Everything except PSUM accumulation (always fp32) and the output runs in
fp16: same PE rate as f32r/bf16 but Fast-Weight-Load-capable weight loads
(fp32-family weights cannot use FWL or column tiling), half the input DMA,
and fp16's 11-bit mantissa is comparable to f32r's TF32-grade rounding
(measured end-to-end: 6.2e-4 vs 4.9e-4 relative, ~170 us faster in a
same-process A/B).
"""

import numpy as np

import concourse.bass as bass
import concourse.mybir as mybir
import concourse.tile as tile
from concourse import bacc, bass_utils

F32 = mybir.dt.float32
F32R = mybir.dt.float32r
BF16 = mybir.dt.bfloat16
F16 = mybir.dt.float16

N_CORES = 8
NG = 16          # number of graphs
GPC = 2          # graphs per core
E = 512
H = 8
D = 64
NEG = -1.0e30

_cache = {}


def _qb_splits(n):
    """Split n into chunks <=512, each >=256 when n permits."""
    out = []
    rem = n
    while rem >= 768:
        out.append(512)
        rem -= 512
    if rem > 512:
        out += [rem - 256, 256]
    elif rem:
        out.append(rem)
    return out


def _build(n_pad, n_q=None, use_f32r=True, reps=1, ablate=None):
    """Build + compile the SPMD device program for a given per-graph pad."""
    if n_q is None:
        n_q = n_pad
    key = (n_pad, n_q, use_f32r, reps, ablate)
    if key in _cache:
        return _cache[key]

    KT = n_pad // 128          # k-tiles per graph
    T_pad = GPC * n_pad        # padded tokens per core
    QBS = _qb_splits(n_pad)    # k-side coverage (layout stride)
    QBSQ = _qb_splits(n_q)     # q-side coverage (queries needed)
    DT = F16

    nc = bacc.Bacc("TRN2", target_bir_lowering=False, debug=False,
                   enable_asserts=False)

    xT_d = nc.dram_tensor("xT", [E, T_pad], DT, kind="ExternalInput")
    wqkv_d = nc.dram_tensor("wqkvT", [E, 3 * E], DT, kind="ExternalInput")
    wp_d = nc.dram_tensor("wpT", [E, E], DT, kind="ExternalInput")
    mask_d = nc.dram_tensor("maskb", [128, GPC * KT], F32, kind="ExternalInput")
    y_d = nc.dram_tensor("y", [T_pad, E], F32, kind="ExternalOutput")

    with tile.TileContext(nc) as tc:
        with (
            tc.tile_pool(name="const", bufs=1) as cpool,
            tc.tile_pool(name="xt", bufs=4) as xtpool,
            tc.tile_pool(name="qkv", bufs=2) as qkvpool,
            tc.tile_pool(name="pt", bufs=4) as ptpool,
            tc.tile_pool(name="small", bufs=3) as smallpool,
            tc.tile_pool(name="ctxn", bufs=3) as ctxnpool,
            tc.tile_pool(name="yout", bufs=3) as ypool,
            tc.tile_pool(name="spsum", bufs=2, space="PSUM") as spsum,
            tc.tile_pool(name="cpsum", bufs=2, space="PSUM") as cpsum,
            tc.tile_pool(name="mpsum", bufs=2, space="PSUM") as mpsum,
        ):
            # ---- constants / weights (resident) ----
            wqkv_sb = cpool.tile([128, 4, 3 * E], DT)   # row-tile e of W^T
            for e in range(4):
                nc.sync.dma_start(wqkv_sb[:, e, :], wqkv_d[128 * e:128 * (e + 1), :])
            wp_sb = cpool.tile([128, 4, E], DT)
            for e in range(4):
                nc.sync.dma_start(wp_sb[:, e, :], wp_d[128 * e:128 * (e + 1), :])
            mask_sb = cpool.tile([128, GPC * KT], F32)
            nc.sync.dma_start(mask_sb[:], mask_d[:])
            ones_sb = cpool.tile([128, 64], F16)
            nc.vector.memset(ones_sb[:], 1.0)

            def load_xt(g, qb0, w):
                xt = xtpool.tile([128, 4, 512], DT, tag="xt", name="xt")
                for e in range(4):
                    nc.sync.dma_start(
                        xt[:, e, :w],
                        xT_d[128 * e:128 * (e + 1),
                             g * n_pad + qb0:g * n_pad + qb0 + w])
                return xt

            def proj_row(xt, r, w):
                """qkT row-tile r for the current q-block held in xt."""
                ps = mpsum.tile([128, 512], F32, tag="mp", name="qkps")
                for e in range(4):
                    nc.tensor.matmul(
                        ps[:, :w],
                        wqkv_sb[:, e, 128 * r:128 * (r + 1)],
                        xt[:, e, :w],
                        start=(e == 0), stop=(e == 3))
                return ps

            for _rep in range(reps):
                for g in range(GPC):
                    qT_sb = qkvpool.tile([128, 4, n_pad], F16, tag="qT",
                                         name="qT")
                    kT_sb = qkvpool.tile([128, 4, n_pad], F16, tag="kT",
                                         name="kT")
                    v_sb = qkvpool.tile([128, KT, E], F16, tag="v", name="v")
                    # ---- pass 1: k^T rows + V natural (full k coverage),
                    #      q^T rows only over the q range ----
                    qb0 = 0
                    for w in QBS:
                        xt = load_xt(g, qb0, w)
                        for r in range(4, 8):
                            ps = proj_row(xt, r, w)
                            nc.vector.tensor_copy(kT_sb[:, r - 4, qb0:qb0 + w],
                                                  ps[:, :w])
                        for tl in range(w // 128):
                            tt = (qb0 + 128 * tl) // 128
                            ps = mpsum.tile([128, 512], F32, tag="mp",
                                            name="vps")
                            for e in range(4):
                                nc.tensor.matmul(
                                    ps[:],
                                    xt[:, e, 128 * tl:128 * (tl + 1)],
                                    wqkv_sb[:, e, 2 * E:3 * E],
                                    start=(e == 0), stop=(e == 3))
                            nc.vector.tensor_copy(v_sb[:, tt, :], ps[:])
                        qb0 += w
                    qb0 = 0
                    for w in QBSQ:
                        xt = load_xt(g, qb0, w)
                        for r in range(4):
                            ps = proj_row(xt, r, w)
                            nc.vector.tensor_copy(qT_sb[:, r, qb0:qb0 + w],
                                                  ps[:, :w])
                        qb0 += w

                    # ---- pass 2: attention + projection per q-block ----
                    qb0 = 0
                    for w in QBSQ:
                        ctxn = ctxnpool.tile([128, 4, 512], DT, tag="ctxn",
                                             name="ctxn")
                        for quad in range(2):
                            ctx_ps = [cpsum.tile([128, 512], F32, tag="cp",
                                                 name=f"ctxps{p}")
                                      for p in range(2)]
                            den_ps = mpsum.tile([128, 512], F32, tag="mp",
                                                name="denps")
                            for kt in range(KT):
                                for pr in range(2):
                                    rt = 2 * quad + pr   # head-pair row-tile
                                    s_ps = spsum.tile([128, 2, 512], F32,
                                                      tag="sp", name="sps")
                                    for j in range(2):
                                        po = 64 * j
                                        nc.tensor.matmul(
                                            s_ps[:, j, :w],
                                            kT_sb[po:po + 64, rt,
                                                  128 * kt:128 * (kt + 1)],
                                            qT_sb[po:po + 64, rt, qb0:qb0 + w],
                                            start=True, stop=True,
                                            tile_position=(po, 0))
                                    pt = ptpool.tile([128, 2, 512], F16,
                                                     tag="pt", name="pt")
                                    if ablate == "noact":
                                        nc.vector.tensor_copy(pt[:, :, :w],
                                                              s_ps[:, :, :w])
                                    else:
                                        nc.scalar.activation(
                                            pt[:, :, :w], s_ps[:, :, :w],
                                            mybir.ActivationFunctionType.Exp,
                                            bias=mask_sb[:, g * KT + kt:
                                                         g * KT + kt + 1],
                                            scale=0.125)
                                    for j in range(2):
                                        h = 4 * quad + 2 * pr + j
                                        i = 2 * pr + j
                                        nc.tensor.matmul(
                                            ctx_ps[pr][64 * j:64 * (j + 1), :w],
                                            v_sb[:, kt, 64 * h:64 * (h + 1)],
                                            pt[:, j, :w],
                                            start=(kt == 0),
                                            stop=(kt == KT - 1),
                                            tile_position=(0, 64 * j))
                                        nc.tensor.matmul(
                                            den_ps[32 * i:32 * i + 1, :w],
                                            ones_sb[:, 0:1],
                                            pt[:, j, :w],
                                            start=(kt == 0),
                                            stop=(kt == KT - 1),
                                            tile_position=(0, 32 * i))
                            # 1/denom rows -> SBUF (bf16, consistent with P)
                            rdenr = smallpool.tile([128, 512], F16,
                                                   tag="rdenr", name="rdenr")
                            with nc.allow_low_precision(reason="f32r rounding"):
                                for i in range(4):
                                    nc.vector.reciprocal(
                                        rdenr[32 * i:32 * i + 1, :w],
                                        den_ps[32 * i:32 * i + 1, :w])
                            # broadcast 1/denom across the 64 d-rows per head
                            for p in range(2):
                                bc_ps = mpsum.tile([128, 512], F32, tag="mp",
                                                   name="bcps")
                                for j in range(2):
                                    i = 2 * p + j
                                    nc.tensor.matmul(
                                        bc_ps[64 * j:64 * (j + 1), :w],
                                        ones_sb[32 * i:32 * i + 1, 0:64],
                                        rdenr[32 * i:32 * i + 1, :w],
                                        start=True, stop=True,
                                        tile_position=(32 * i, 64 * j))
                                bc_sb = smallpool.tile([128, 512], F32,
                                                       tag="bcs", name="bcsb")
                                nc.vector.tensor_copy(bc_sb[:, :w],
                                                      bc_ps[:, :w])
                                nc.vector.tensor_mul(
                                    ctxn[:, 2 * quad + p, :w],
                                    ctx_ps[p][:, :w], bc_sb[:, :w])
                        # ---- fused out projection for this q-block ----
                        tl0 = 0
                        while tl0 < w:
                            ts_ = min(128, w - tl0)
                            yps = mpsum.tile([128, 512], F32, tag="mp",
                                             name="yps")
                            for e in range(4):
                                nc.tensor.matmul(
                                    yps[:ts_, :],
                                    ctxn[:, e, tl0:tl0 + ts_],
                                    wp_sb[:, e, :],
                                    start=(e == 0), stop=(e == 3))
                            ysb = ypool.tile([128, 512], F32, tag="y",
                                             name="ysb")
                            nc.vector.tensor_copy(ysb[:ts_, :], yps[:ts_, :])
                            row0 = g * n_pad + qb0 + tl0
                            nc.sync.dma_start(y_d[row0:row0 + ts_, :],
                                              ysb[:ts_, :])
                            tl0 += ts_
                        qb0 += w

    nc.compile()
    _cache[key] = (nc, KT, T_pad, QBS)
    return _cache[key]


def kernel(x, batch, in_proj_w, in_proj_b, out_proj_w, out_proj_b,
           lin_w, lin_b):
    x = np.ascontiguousarray(np.asarray(x, dtype=np.float32))
    b = np.asarray(batch).astype(np.int64)
    in_proj_w = np.asarray(in_proj_w, dtype=np.float32)
    in_proj_b = np.asarray(in_proj_b, dtype=np.float32)
    out_proj_w = np.asarray(out_proj_w, dtype=np.float32)
    out_proj_b = np.asarray(out_proj_b, dtype=np.float32)
    lin_w = np.asarray(lin_w, dtype=np.float32)
    lin_b = np.asarray(lin_b, dtype=np.float32)

    T = x.shape[0]
    counts = np.bincount(b, minlength=NG)
    assert counts.sum() == T and len(counts) == NG
    offsets = np.concatenate([[0], np.cumsum(counts)[:-1]])
    n_pad = ((int(counts.max()) + 127) // 128) * 128
    n_q = ((int(counts.max()) + 63) // 64) * 64   # q rows actually needed

    nc, KT, T_pad, _ = _build(n_pad, n_q)

    # host-side fused weights (shared across cores), fp16 on device
    wqkvT = np.ascontiguousarray(in_proj_w.T).astype(np.float16)   # [512,1536]
    wpT = np.ascontiguousarray(out_proj_w.T @ lin_w.T).astype(np.float16)
    # biases are zero in this problem; assert so silently-wrong results
    # can't slip through if the harness ever changes them.
    assert not in_proj_b.any() and not out_proj_b.any() \
        and not lin_b.any() and not (out_proj_b @ lin_w.T + lin_b).any(), \
        "nonzero biases not supported by this build"

    in_maps = []
    for c in range(N_CORES):
        xT = np.zeros((E, T_pad), np.float16)
        maskb = np.full((128, GPC * KT), NEG, np.float32)
        for s in range(GPC):
            g = GPC * c + s
            n = int(counts[g])
            o = int(offsets[g])
            xT[:, s * n_pad:s * n_pad + n] = x[o:o + n].T.astype(np.float16)
            for kt in range(KT):
                valid = min(max(n - 128 * kt, 0), 128)
                maskb[:valid, s * KT + kt] = -8.0
        in_maps.append({
            "xT": np.ascontiguousarray(xT),
            "wqkvT": wqkvT,
            "wpT": wpT,
            "maskb": np.ascontiguousarray(maskb),
        })

    res = bass_utils.run_bass_kernel_spmd(nc, in_maps, core_ids=list(range(N_CORES)))

    out = np.empty((T, E), np.float32)
    for c in range(N_CORES):
        yc = res.results[c]["y"]
        for s in range(GPC):
            g = GPC * c + s
            n = int(counts[g])
            o = int(offsets[g])
            out[o:o + n] = yc[s * n_pad:s * n_pad + n]
    return out



# revision 4
# speedup vs baseline: 1.3474x; 1.3474x over previous
"""Trainium2 Bass kernel for CrossGraphAttention (ragged per-graph MHA + linear).

Strategy: data-parallel over graphs (2 graphs per core x 8 cores). All graphs
padded to a common n_pad (multiple of 128). Per core the device program:
  1. QKV projection: all qk^T row-tiles + V natural per q-block (pass 1),
     then pure attention + output projection per q-block (pass 2).
  2. Scores computed TRANSPOSED (S^T[k, q]) per head-pair so the softmax
     denominator is a ones-vector matmul (partition reductions are
     PE-friendly); exp fused with the PSUM->SBUF eviction on the scalar
     engine ([128, 2*w] per instruction), with key-padding masking via a
     per-partition bias of -1e30 (exp -> 0). Head-pair score tiles are
     double-buffered in PSUM so PE and ACT pipeline across k-tiles. The
     query iteration covers only ceil(max_graph/64)*64 columns, not the
     128-multiple k-layout stride.
  3. ctx^T accumulated over k-tiles in PSUM (2 heads packed per bank via
     column tiling, fp16 operands; the exp bias also folds in a fixed -8
     offset so probabilities stay in fp16 range — it cancels in softmax);
     normalization by 1/denom applied via a rank-1 broadcast matmul +
     vector multiply.
  4. Fused output projection y = ctx @ (lin_w @ out_proj_w)^T.
Everything except PSUM accumulation (always fp32) and the output runs in
fp16: same PE rate as f32r/bf16 but Fast-Weight-Load-capable weight loads
(fp32-family weights cannot use FWL or column tiling), half the input DMA,
and fp16's 11-bit mantissa is comparable to f32r's TF32-grade rounding
(measured end-to-end: 6.2e-4 vs 4.9e-4 relative, ~170 us faster in a
same-process A/B).
"""

import numpy as np

import concourse.bass as bass
import concourse.mybir as mybir
import concourse.tile as tile
from concourse import bacc, bass_utils

F32 = mybir.dt.float32
F32R = mybir.dt.float32r
BF16 = mybir.dt.bfloat16
F16 = mybir.dt.float16

N_CORES = 8
NG = 16          # number of graphs
GPC = 2          # graphs per core
E = 512
H = 8
D = 64
NEG = -1.0e30

_cache = {}


def _qb_splits(n):
    """Split n into chunks <=512, each >=256 when n permits."""
    out = []
    rem = n
    while rem >= 768:
        out.append(512)
        rem -= 512
    if rem > 512:
        out += [rem - 256, 256]
    elif rem:
        out.append(rem)
    return out


def _build(n_pad, n_q=None, use_f32r=True, reps=1, ablate=None):
    """Build + compile the SPMD device program for a given per-graph pad."""
    if n_q is None:
        n_q = n_pad
    key = (n_pad, n_q, use_f32r, reps, ablate)
    if key in _cache:
        return _cache[key]

    KT = n_pad // 128          # k-tiles per graph
    T_pad = GPC * n_pad        # padded tokens per core
    QBS = _qb_splits(n_pad)    # k-side coverage (layout stride)
    QBSQ = _qb_splits(n_q)     # q-side coverage (queries needed)
    DT = F16

    nc = bacc.Bacc("TRN2", target_bir_lowering=False, debug=False,
                   enable_asserts=False)

    xT_d = nc.dram_tensor("xT", [E, T_pad], DT, kind="ExternalInput")
    wqkv_d = nc.dram_tensor("wqkvT", [E, 3 * E], DT, kind="ExternalInput")
    wp_d = nc.dram_tensor("wpT", [E, E], DT, kind="ExternalInput")
    mask_d = nc.dram_tensor("maskb", [128, GPC * KT], F32, kind="ExternalInput")
    y_d = nc.dram_tensor("y", [T_pad, E], F16, kind="ExternalOutput")

    with tile.TileContext(nc) as tc:
        with (
            tc.tile_pool(name="const", bufs=1) as cpool,
            tc.tile_pool(name="xt", bufs=4) as xtpool,
            tc.tile_pool(name="qkv", bufs=2) as qkvpool,
            tc.tile_pool(name="pt", bufs=4) as ptpool,
            tc.tile_pool(name="small", bufs=3) as smallpool,
            tc.tile_pool(name="ctxn", bufs=3) as ctxnpool,
            tc.tile_pool(name="yout", bufs=3) as ypool,
            tc.tile_pool(name="spsum", bufs=2, space="PSUM") as spsum,
            tc.tile_pool(name="cpsum", bufs=2, space="PSUM") as cpsum,
            tc.tile_pool(name="mpsum", bufs=2, space="PSUM") as mpsum,
        ):
            # ---- constants / weights (resident) ----
            wqkv_sb = cpool.tile([128, 4, 3 * E], DT)   # row-tile e of W^T
            for e in range(4):
                nc.sync.dma_start(wqkv_sb[:, e, :], wqkv_d[128 * e:128 * (e + 1), :])
            wp_sb = cpool.tile([128, 4, E], DT)
            for e in range(4):
                nc.sync.dma_start(wp_sb[:, e, :], wp_d[128 * e:128 * (e + 1), :])
            mask_sb = cpool.tile([128, GPC * KT], F32)
            nc.sync.dma_start(mask_sb[:], mask_d[:])
            ones_sb = cpool.tile([128, 64], F16)
            nc.vector.memset(ones_sb[:], 1.0)

            def load_xt(g, qb0, w):
                xt = xtpool.tile([128, 4, 512], DT, tag="xt", name="xt")
                for e in range(4):
                    nc.sync.dma_start(
                        xt[:, e, :w],
                        xT_d[128 * e:128 * (e + 1),
                             g * n_pad + qb0:g * n_pad + qb0 + w])
                return xt

            def proj_row(xt, r, w):
                """qkT row-tile r for the current q-block held in xt."""
                ps = mpsum.tile([128, 512], F32, tag="mp", name="qkps")
                for e in range(4):
                    nc.tensor.matmul(
                        ps[:, :w],
                        wqkv_sb[:, e, 128 * r:128 * (r + 1)],
                        xt[:, e, :w],
                        start=(e == 0), stop=(e == 3))
                return ps

            for _rep in range(reps):
                for g in range(GPC):
                    qT_sb = qkvpool.tile([128, 4, n_pad], F16, tag="qT",
                                         name="qT")
                    kT_sb = qkvpool.tile([128, 4, n_pad], F16, tag="kT",
                                         name="kT")
                    v_sb = qkvpool.tile([128, KT, E], F16, tag="v", name="v")
                    # ---- pass 1: k^T rows + V natural (full k coverage),
                    #      q^T rows only over the q range ----
                    qb0 = 0
                    for w in QBS:
                        xt = load_xt(g, qb0, w)
                        for r in range(4, 8):
                            ps = proj_row(xt, r, w)
                            nc.vector.tensor_copy(kT_sb[:, r - 4, qb0:qb0 + w],
                                                  ps[:, :w])
                        for tl in range(w // 128):
                            tt = (qb0 + 128 * tl) // 128
                            ps = mpsum.tile([128, 512], F32, tag="mp",
                                            name="vps")
                            for e in range(4):
                                nc.tensor.matmul(
                                    ps[:],
                                    xt[:, e, 128 * tl:128 * (tl + 1)],
                                    wqkv_sb[:, e, 2 * E:3 * E],
                                    start=(e == 0), stop=(e == 3))
                            nc.vector.tensor_copy(v_sb[:, tt, :], ps[:])
                        qb0 += w
                    qb0 = 0
                    for w in QBSQ:
                        xt = load_xt(g, qb0, w)
                        for r in range(4):
                            ps = proj_row(xt, r, w)
                            nc.vector.tensor_copy(qT_sb[:, r, qb0:qb0 + w],
                                                  ps[:, :w])
                        qb0 += w

                    # ---- pass 2: attention + projection per q-block ----
                    qb0 = 0
                    for w in QBSQ:
                        ctxn = ctxnpool.tile([128, 4, 512], DT, tag="ctxn",
                                             name="ctxn")
                        for quad in range(2):
                            ctx_ps = [cpsum.tile([128, 512], F32, tag="cp",
                                                 name=f"ctxps{p}")
                                      for p in range(2)]
                            den_ps = mpsum.tile([128, 512], F32, tag="mp",
                                                name="denps")
                            for kt in range(KT):
                                for pr in range(2):
                                    rt = 2 * quad + pr   # head-pair row-tile
                                    s_ps = spsum.tile([128, 2, 512], F32,
                                                      tag="sp", name="sps")
                                    for j in range(2):
                                        po = 64 * j
                                        nc.tensor.matmul(
                                            s_ps[:, j, :w],
                                            kT_sb[po:po + 64, rt,
                                                  128 * kt:128 * (kt + 1)],
                                            qT_sb[po:po + 64, rt, qb0:qb0 + w],
                                            start=True, stop=True,
                                            tile_position=(po, 0))
                                    pt = ptpool.tile([128, 2, 512], F16,
                                                     tag="pt", name="pt")
                                    if ablate == "noact":
                                        nc.vector.tensor_copy(pt[:, :, :w],
                                                              s_ps[:, :, :w])
                                    else:
                                        nc.scalar.activation(
                                            pt[:, :, :w], s_ps[:, :, :w],
                                            mybir.ActivationFunctionType.Exp,
                                            bias=mask_sb[:, g * KT + kt:
                                                         g * KT + kt + 1],
                                            scale=0.125)
                                    for j in range(2):
                                        h = 4 * quad + 2 * pr + j
                                        i = 2 * pr + j
                                        nc.tensor.matmul(
                                            ctx_ps[pr][64 * j:64 * (j + 1), :w],
                                            v_sb[:, kt, 64 * h:64 * (h + 1)],
                                            pt[:, j, :w],
                                            start=(kt == 0),
                                            stop=(kt == KT - 1),
                                            tile_position=(0, 64 * j))
                                        nc.tensor.matmul(
                                            den_ps[32 * i:32 * i + 1, :w],
                                            ones_sb[:, 0:1],
                                            pt[:, j, :w],
                                            start=(kt == 0),
                                            stop=(kt == KT - 1),
                                            tile_position=(0, 32 * i))
                            # 1/denom rows -> SBUF (bf16, consistent with P)
                            rdenr = smallpool.tile([128, 512], F16,
                                                   tag="rdenr", name="rdenr")
                            with nc.allow_low_precision(reason="f32r rounding"):
                                for i in range(4):
                                    nc.vector.reciprocal(
                                        rdenr[32 * i:32 * i + 1, :w],
                                        den_ps[32 * i:32 * i + 1, :w])
                            # broadcast 1/denom across the 64 d-rows per head
                            for p in range(2):
                                bc_ps = mpsum.tile([128, 512], F32, tag="mp",
                                                   name="bcps")
                                for j in range(2):
                                    i = 2 * p + j
                                    nc.tensor.matmul(
                                        bc_ps[64 * j:64 * (j + 1), :w],
                                        ones_sb[32 * i:32 * i + 1, 0:64],
                                        rdenr[32 * i:32 * i + 1, :w],
                                        start=True, stop=True,
                                        tile_position=(32 * i, 64 * j))
                                bc_sb = smallpool.tile([128, 512], F32,
                                                       tag="bcs", name="bcsb")
                                nc.vector.tensor_copy(bc_sb[:, :w],
                                                      bc_ps[:, :w])
                                nc.vector.tensor_mul(
                                    ctxn[:, 2 * quad + p, :w],
                                    ctx_ps[p][:, :w], bc_sb[:, :w])
                        # ---- fused out projection for this q-block ----
                        tl0 = 0
                        while tl0 < w:
                            ts_ = min(128, w - tl0)
                            yps = mpsum.tile([128, 512], F32, tag="mp",
                                             name="yps")
                            for e in range(4):
                                nc.tensor.matmul(
                                    yps[:ts_, :],
                                    ctxn[:, e, tl0:tl0 + ts_],
                                    wp_sb[:, e, :],
                                    start=(e == 0), stop=(e == 3))
                            ysb = ypool.tile([128, 512], F16, tag="y",
                                             name="ysb")
                            nc.vector.tensor_copy(ysb[:ts_, :], yps[:ts_, :])
                            row0 = g * n_pad + qb0 + tl0
                            nc.sync.dma_start(y_d[row0:row0 + ts_, :],
                                              ysb[:ts_, :])
                            tl0 += ts_
                        qb0 += w

    nc.compile()
    _cache[key] = (nc, KT, T_pad, QBS)
    return _cache[key]


def kernel(x, batch, in_proj_w, in_proj_b, out_proj_w, out_proj_b,
           lin_w, lin_b):
    x = np.ascontiguousarray(np.asarray(x, dtype=np.float32))
    b = np.asarray(batch).astype(np.int64)
    in_proj_w = np.asarray(in_proj_w, dtype=np.float32)
    in_proj_b = np.asarray(in_proj_b, dtype=np.float32)
    out_proj_w = np.asarray(out_proj_w, dtype=np.float32)
    out_proj_b = np.asarray(out_proj_b, dtype=np.float32)
    lin_w = np.asarray(lin_w, dtype=np.float32)
    lin_b = np.asarray(lin_b, dtype=np.float32)

    T = x.shape[0]
    counts = np.bincount(b, minlength=NG)
    assert counts.sum() == T and len(counts) == NG
    offsets = np.concatenate([[0], np.cumsum(counts)[:-1]])
    n_pad = ((int(counts.max()) + 127) // 128) * 128
    n_q = ((int(counts.max()) + 63) // 64) * 64   # q rows actually needed

    nc, KT, T_pad, _ = _build(n_pad, n_q)

    # host-side fused weights (shared across cores), fp16 on device
    wqkvT = np.ascontiguousarray(in_proj_w.T).astype(np.float16)   # [512,1536]
    wpT = np.ascontiguousarray(out_proj_w.T @ lin_w.T).astype(np.float16)
    # biases are zero in this problem; assert so silently-wrong results
    # can't slip through if the harness ever changes them.
    assert not in_proj_b.any() and not out_proj_b.any() \
        and not lin_b.any() and not (out_proj_b @ lin_w.T + lin_b).any(), \
        "nonzero biases not supported by this build"

    in_maps = []
    for c in range(N_CORES):
        xT = np.zeros((E, T_pad), np.float16)
        maskb = np.full((128, GPC * KT), NEG, np.float32)
        for s in range(GPC):
            g = GPC * c + s
            n = int(counts[g])
            o = int(offsets[g])
            xT[:, s * n_pad:s * n_pad + n] = x[o:o + n].T.astype(np.float16)
            for kt in range(KT):
                valid = min(max(n - 128 * kt, 0), 128)
                maskb[:valid, s * KT + kt] = -8.0
        in_maps.append({
            "xT": np.ascontiguousarray(xT),
            "wqkvT": wqkvT,
            "wpT": wpT,
            "maskb": np.ascontiguousarray(maskb),
        })

    res = bass_utils.run_bass_kernel_spmd(nc, in_maps, core_ids=list(range(N_CORES)))

    out = np.empty((T, E), np.float32)
    for c in range(N_CORES):
        yc = res.results[c]["y"]
        for s in range(GPC):
            g = GPC * c + s
            n = int(counts[g])
            o = int(offsets[g])
            out[o:o + n] = yc[s * n_pad:s * n_pad + n].astype(np.float32)
    return out



# revision 5
# speedup vs baseline: 1.5371x; 1.1408x over previous
"""Trainium2 Bass kernel for CrossGraphAttention (ragged per-graph MHA + linear).

Strategy: data-parallel over graphs (2 graphs per core x 8 cores). All graphs
padded to a common n_pad (multiple of 128). Per core the device program:
  1. QKV projection: all qk^T row-tiles + V natural per q-block (pass 1),
     then pure attention + output projection per q-block (pass 2).
  2. Scores computed TRANSPOSED (S^T[k, q]) per head-pair so the softmax
     denominator is a ones-vector matmul (partition reductions are
     PE-friendly); exp fused with the PSUM->SBUF eviction on the scalar
     engine ([128, 2*w] per instruction), with key-padding masking via a
     per-partition bias of -60000 (exp -> 0). Head-pair score tiles are
     double-buffered in PSUM so PE and ACT pipeline across k-tiles. The
     query iteration covers only ceil(max_graph/64)*64 columns, not the
     128-multiple k-layout stride.
  3. ctx^T accumulated over k-tiles in PSUM (2 heads packed per bank via
     column tiling, fp16 operands; the exp bias also folds in a fixed -8
     offset so probabilities stay in fp16 range — it cancels in softmax);
     normalization by 1/denom applied via a rank-1 broadcast matmul +
     vector multiply.
  4. Fused output projection y = ctx @ (lin_w @ out_proj_w)^T.
Everything except PSUM accumulation (always fp32) runs in fp16, including
the DRAM I/O: fp16 halves relay/DMA traffic and its 11-bit mantissa keeps
end-to-end error ~7e-4 (tolerance 2e-2). All per-core inputs (x^T, packed
QKV weights, fused output weights, key-padding mask biases) are fused into
ONE DRAM tensor so the host->device path ships a single buffer per core;
the output y is a single fp16 tensor.
"""

import numpy as np

import concourse.bass as bass
import concourse.mybir as mybir
import concourse.tile as tile
from concourse import bacc, bass_utils

F32 = mybir.dt.float32
F16 = mybir.dt.float16

N_CORES = 8
NG = 16          # number of graphs
GPC = 2          # graphs per core
E = 512
H = 8
D = 64
NEG = -60000.0   # exp(scale*s + NEG) == 0 in fp32; representable in fp16

_cache = {}


def _qb_splits(n):
    """Split n into chunks <=512, each >=256 when n permits."""
    out = []
    rem = n
    while rem >= 768:
        out.append(512)
        rem -= 512
    if rem > 512:
        out += [rem - 256, 256]
    elif rem:
        out.append(rem)
    return out


def _layout(n_pad):
    """Column layout of the fused per-core input blob [128, NCOL] fp16."""
    T_pad = GPC * n_pad
    KT = n_pad // 128
    xoff = 0                      # 4 chunks of T_pad cols (rows 128e of x^T)
    woff = xoff + 4 * T_pad       # 4 chunks of 1536 cols (rows 128e of Wqkv^T)
    poff = woff + 4 * 3 * E       # 4 chunks of 512 cols (rows 128e of Wp^T)
    moff = poff + 4 * E           # GPC*KT cols of per-partition mask bias
    ncol = moff + GPC * KT
    return T_pad, KT, xoff, woff, poff, moff, ncol


def _build(n_pad, n_q=None, reps=1):
    """Build + compile the SPMD device program for a given per-graph pad."""
    if n_q is None:
        n_q = n_pad
    key = (n_pad, n_q, reps)
    if key in _cache:
        return _cache[key]

    T_pad, KT, XOFF, WOFF, POFF, MOFF, NCOL = _layout(n_pad)
    QBS = _qb_splits(n_pad)    # k-side coverage (layout stride)
    QBSQ = _qb_splits(n_q)     # q-side coverage (queries needed)
    DT = F16

    nc = bacc.Bacc("TRN2", target_bir_lowering=False, debug=False,
                   enable_asserts=False)

    blob_d = nc.dram_tensor("blob", [128, NCOL], DT, kind="ExternalInput")
    y_d = nc.dram_tensor("y", [T_pad, E], F16, kind="ExternalOutput")

    with tile.TileContext(nc) as tc:
        with (
            tc.tile_pool(name="const", bufs=1) as cpool,
            tc.tile_pool(name="xt", bufs=4) as xtpool,
            tc.tile_pool(name="qkv", bufs=2) as qkvpool,
            tc.tile_pool(name="pt", bufs=4) as ptpool,
            tc.tile_pool(name="small", bufs=3) as smallpool,
            tc.tile_pool(name="ctxn", bufs=3) as ctxnpool,
            tc.tile_pool(name="yout", bufs=3) as ypool,
            tc.tile_pool(name="spsum", bufs=2, space="PSUM") as spsum,
            tc.tile_pool(name="cpsum", bufs=2, space="PSUM") as cpsum,
            tc.tile_pool(name="mpsum", bufs=2, space="PSUM") as mpsum,
        ):
            # ---- constants / weights (resident) ----
            wqkv_sb = cpool.tile([128, 4, 3 * E], DT)   # row-tile e of W^T
            for e in range(4):
                nc.sync.dma_start(wqkv_sb[:, e, :],
                                  blob_d[:, WOFF + 3 * E * e:
                                         WOFF + 3 * E * (e + 1)])
            wp_sb = cpool.tile([128, 4, E], DT)
            for e in range(4):
                nc.sync.dma_start(wp_sb[:, e, :],
                                  blob_d[:, POFF + E * e:POFF + E * (e + 1)])
            mask_sb = cpool.tile([128, GPC * KT], DT)
            nc.sync.dma_start(mask_sb[:], blob_d[:, MOFF:MOFF + GPC * KT])
            ones_sb = cpool.tile([128, 64], F16)
            nc.vector.memset(ones_sb[:], 1.0)

            def load_xt(g, qb0, w):
                xt = xtpool.tile([128, 4, 512], DT, tag="xt", name="xt")
                c0 = g * n_pad + qb0
                for e in range(4):
                    nc.sync.dma_start(
                        xt[:, e, :w],
                        blob_d[:, XOFF + e * T_pad + c0:
                               XOFF + e * T_pad + c0 + w])
                return xt

            def proj_row(xt, r, w):
                """qkT row-tile r for the current q-block held in xt."""
                ps = mpsum.tile([128, 512], F32, tag="mp", name="qkps")
                for e in range(4):
                    nc.tensor.matmul(
                        ps[:, :w],
                        wqkv_sb[:, e, 128 * r:128 * (r + 1)],
                        xt[:, e, :w],
                        start=(e == 0), stop=(e == 3))
                return ps

            for _rep in range(reps):
                for g in range(GPC):
                    qT_sb = qkvpool.tile([128, 4, n_pad], F16, tag="qT",
                                         name="qT")
                    kT_sb = qkvpool.tile([128, 4, n_pad], F16, tag="kT",
                                         name="kT")
                    v_sb = qkvpool.tile([128, KT, E], F16, tag="v", name="v")
                    # ---- pass 1: k^T rows + V natural (full k coverage),
                    #      q^T rows only over the q range ----
                    qb0 = 0
                    for w in QBS:
                        xt = load_xt(g, qb0, w)
                        for r in range(4, 8):
                            ps = proj_row(xt, r, w)
                            nc.vector.tensor_copy(kT_sb[:, r - 4, qb0:qb0 + w],
                                                  ps[:, :w])
                        for tl in range(w // 128):
                            tt = (qb0 + 128 * tl) // 128
                            ps = mpsum.tile([128, 512], F32, tag="mp",
                                            name="vps")
                            for e in range(4):
                                nc.tensor.matmul(
                                    ps[:],
                                    xt[:, e, 128 * tl:128 * (tl + 1)],
                                    wqkv_sb[:, e, 2 * E:3 * E],
                                    start=(e == 0), stop=(e == 3))
                            nc.vector.tensor_copy(v_sb[:, tt, :], ps[:])
                        qb0 += w
                    qb0 = 0
                    for w in QBSQ:
                        xt = load_xt(g, qb0, w)
                        for r in range(4):
                            ps = proj_row(xt, r, w)
                            nc.vector.tensor_copy(qT_sb[:, r, qb0:qb0 + w],
                                                  ps[:, :w])
                        qb0 += w

                    # ---- pass 2: attention + projection per q-block ----
                    qb0 = 0
                    for w in QBSQ:
                        ctxn = ctxnpool.tile([128, 4, 512], DT, tag="ctxn",
                                             name="ctxn")
                        for quad in range(2):
                            ctx_ps = [cpsum.tile([128, 512], F32, tag="cp",
                                                 name=f"ctxps{p}")
                                      for p in range(2)]
                            den_ps = mpsum.tile([128, 512], F32, tag="mp",
                                                name="denps")
                            for kt in range(KT):
                                for pr in range(2):
                                    rt = 2 * quad + pr   # head-pair row-tile
                                    s_ps = spsum.tile([128, 2, 512], F32,
                                                      tag="sp", name="sps")
                                    for j in range(2):
                                        po = 64 * j
                                        nc.tensor.matmul(
                                            s_ps[:, j, :w],
                                            kT_sb[po:po + 64, rt,
                                                  128 * kt:128 * (kt + 1)],
                                            qT_sb[po:po + 64, rt, qb0:qb0 + w],
                                            start=True, stop=True,
                                            tile_position=(po, 0))
                                    pt = ptpool.tile([128, 2, 512], F16,
                                                     tag="pt", name="pt")
                                    nc.scalar.activation(
                                        pt[:, :, :w], s_ps[:, :, :w],
                                        mybir.ActivationFunctionType.Exp,
                                        bias=mask_sb[:, g * KT + kt:
                                                     g * KT + kt + 1],
                                        scale=0.125)
                                    for j in range(2):
                                        h = 4 * quad + 2 * pr + j
                                        i = 2 * pr + j
                                        nc.tensor.matmul(
                                            ctx_ps[pr][64 * j:64 * (j + 1), :w],
                                            v_sb[:, kt, 64 * h:64 * (h + 1)],
                                            pt[:, j, :w],
                                            start=(kt == 0),
                                            stop=(kt == KT - 1),
                                            tile_position=(0, 64 * j))
                                        nc.tensor.matmul(
                                            den_ps[32 * i:32 * i + 1, :w],
                                            ones_sb[:, 0:1],
                                            pt[:, j, :w],
                                            start=(kt == 0),
                                            stop=(kt == KT - 1),
                                            tile_position=(0, 32 * i))
                            # 1/denom rows -> SBUF (fp16, consistent with P)
                            rdenr = smallpool.tile([128, 512], F16,
                                                   tag="rdenr", name="rdenr")
                            with nc.allow_low_precision(reason="f32r rounding"):
                                for i in range(4):
                                    nc.vector.reciprocal(
                                        rdenr[32 * i:32 * i + 1, :w],
                                        den_ps[32 * i:32 * i + 1, :w])
                            # broadcast 1/denom across the 64 d-rows per head
                            for p in range(2):
                                bc_ps = mpsum.tile([128, 512], F32, tag="mp",
                                                   name="bcps")
                                for j in range(2):
                                    i = 2 * p + j
                                    nc.tensor.matmul(
                                        bc_ps[64 * j:64 * (j + 1), :w],
                                        ones_sb[32 * i:32 * i + 1, 0:64],
                                        rdenr[32 * i:32 * i + 1, :w],
                                        start=True, stop=True,
                                        tile_position=(32 * i, 64 * j))
                                bc_sb = smallpool.tile([128, 512], F32,
                                                       tag="bcs", name="bcsb")
                                nc.vector.tensor_copy(bc_sb[:, :w],
                                                      bc_ps[:, :w])
                                nc.vector.tensor_mul(
                                    ctxn[:, 2 * quad + p, :w],
                                    ctx_ps[p][:, :w], bc_sb[:, :w])
                        # ---- fused out projection for this q-block ----
                        tl0 = 0
                        while tl0 < w:
                            ts_ = min(128, w - tl0)
                            yps = mpsum.tile([128, 512], F32, tag="mp",
                                             name="yps")
                            for e in range(4):
                                nc.tensor.matmul(
                                    yps[:ts_, :],
                                    ctxn[:, e, tl0:tl0 + ts_],
                                    wp_sb[:, e, :],
                                    start=(e == 0), stop=(e == 3))
                            ysb = ypool.tile([128, 512], F16, tag="y",
                                             name="ysb")
                            nc.vector.tensor_copy(ysb[:ts_, :], yps[:ts_, :])
                            row0 = g * n_pad + qb0 + tl0
                            nc.sync.dma_start(y_d[row0:row0 + ts_, :],
                                              ysb[:ts_, :])
                            tl0 += ts_
                        qb0 += w

    nc.compile()
    _cache[key] = (nc, KT, T_pad, QBS)
    return _cache[key]


def kernel(x, batch, in_proj_w, in_proj_b, out_proj_w, out_proj_b,
           lin_w, lin_b):
    x = np.ascontiguousarray(np.asarray(x, dtype=np.float32))
    b = np.asarray(batch).astype(np.int64)
    in_proj_w = np.asarray(in_proj_w, dtype=np.float32)
    in_proj_b = np.asarray(in_proj_b, dtype=np.float32)
    out_proj_w = np.asarray(out_proj_w, dtype=np.float32)
    out_proj_b = np.asarray(out_proj_b, dtype=np.float32)
    lin_w = np.asarray(lin_w, dtype=np.float32)
    lin_b = np.asarray(lin_b, dtype=np.float32)

    T = x.shape[0]
    counts = np.bincount(b, minlength=NG)
    assert counts.sum() == T and len(counts) == NG
    offsets = np.concatenate([[0], np.cumsum(counts)[:-1]])
    n_pad = ((int(counts.max()) + 127) // 128) * 128
    n_q = ((int(counts.max()) + 63) // 64) * 64   # q rows actually needed

    nc, KT, T_pad, _ = _build(n_pad, n_q)
    _, _, XOFF, WOFF, POFF, MOFF, NCOL = _layout(n_pad)

    # host-side fused weights (shared across cores), fp16 on device
    wqkvT = np.ascontiguousarray(in_proj_w.T).astype(np.float16)   # [512,1536]
    wpT = np.ascontiguousarray(out_proj_w.T @ lin_w.T).astype(np.float16)
    # biases are zero in this problem; assert so silently-wrong results
    # can't slip through if the harness ever changes them.
    assert not in_proj_b.any() and not out_proj_b.any() \
        and not lin_b.any() and not (out_proj_b @ lin_w.T + lin_b).any(), \
        "nonzero biases not supported by this build"

    x16 = x.astype(np.float16)
    wsec = np.empty((128, 4 * 4 * E), np.float16)   # W chunks then P chunks
    for e in range(4):
        wsec[:, 3 * E * e:3 * E * (e + 1)] = wqkvT[128 * e:128 * (e + 1), :]
        wsec[:, 12 * E + E * e:12 * E + E * (e + 1)] = \
            wpT[128 * e:128 * (e + 1), :]

    in_maps = []
    for c in range(N_CORES):
        blob = np.zeros((128, NCOL), np.float16)
        blob[:, WOFF:MOFF] = wsec
        blob[:, MOFF:] = NEG
        for s in range(GPC):
            g = GPC * c + s
            n = int(counts[g])
            o = int(offsets[g])
            xg = x16[o:o + n]
            for e in range(4):
                c0 = XOFF + e * T_pad + s * n_pad
                blob[:, c0:c0 + n] = xg[:, 128 * e:128 * (e + 1)].T
            for kt in range(KT):
                valid = min(max(n - 128 * kt, 0), 128)
                blob[:valid, MOFF + s * KT + kt] = -8.0
        in_maps.append({"blob": blob})

    res = bass_utils.run_bass_kernel_spmd(nc, in_maps,
                                          core_ids=list(range(N_CORES)))

    out = np.empty((T, E), np.float32)
    for c in range(N_CORES):
        yc = res.results[c]["y"]
        for s in range(GPC):
            g = GPC * c + s
            n = int(counts[g])
            o = int(offsets[g])
            out[o:o + n] = yc[s * n_pad:s * n_pad + n].astype(np.float32)
    return out
